# revision 1
# baseline (speedup 1.0000x reference)
"""Trainium2 Bass kernel for nn_Decoder_5334349382400.

3-layer transformer decoder (self-attn + cross-attn + FFN + LN) with
norm-softmax pooling and a 2-class head, batch=1, seq 2048, hid 512.

Sharding: sequence-parallel over 8 NeuronCores (256 tokens/core).
 - All per-token work (projections, FFN, LN, softmax rows) is local.
 - Self-attention K/V are computed locally per-core and AllGathered
   (bf16) once per layer; cross-attention K/V depend only on `src`, so
   they are computed+gathered once for all 3 layers up front.
 - Final pooling uses a tiny AllReduce of [wsum(512) | denom(1)].

Layout: activations live transposed in SBUF, xT[feat(part), tok(free)],
packed [128, 4, 256] (feat chunk-major). Every projection is then
out = W[in,out] as lhsT (stationary), xT as moving -> (x@W)^T, with
per-partition biases applied on the ScalarE evacuation. Attention
scores are computed transposed (scoresT[k, q]) so A@V needs no
transpose; the softmax denominator rides as a ones-column appended to V.
"""

import sys

sys.path.insert(0, "/opt/trn_rl_repo")

import numpy as np
import ml_dtypes

import concourse.bass as bass
import concourse.mybir as mybir
import concourse.tile as tile
from concourse import bacc, bass_utils

BF16 = ml_dtypes.bfloat16
F32 = mybir.dt.float32
BF = mybir.dt.bfloat16
AX = mybir.AxisListType
ALU = mybir.AluOpType
ACTF = mybir.ActivationFunctionType

C = 8          # cores
T = 2048       # tokens
TC = T // C    # tokens per core (256)
D = 512        # hidden
H = 8          # heads
HD = 64        # head dim
PF = 2048      # ffn dim
L = 3          # layers
ATOM = 64      # trg feature dim
NC4 = D // 128   # 4 feature chunks
NPF = PF // 128  # 16
NKT = T // 128   # 16 key tiles
EPS = 1e-5

# bias-pack column map
FT_B = 0
LBASE = 4
LSTRIDE = 44
SA_BQ, SA_BO, EA_BQ, EA_BO, B1, B2, LNG, LNB = 0, 4, 8, 12, 16, 32, 36, 40
FC1_B = LBASE + L * LSTRIDE          # 136
FC2_B = FC1_B + 2                    # 138
NCOL = FC2_B + 1                     # 139


def _bcol(l, off):
    return LBASE + l * LSTRIDE + off


def build_program():
    nc = bacc.Bacc("TRN2", target_bir_lowering=False, debug=False,
                   enable_asserts=True, num_devices=C)

    # ---- DRAM I/O ----
    t_trgT = nc.dram_tensor("trgT", [ATOM, TC], BF, kind="ExternalInput")
    t_srcT = nc.dram_tensor("srcT", [D, TC], BF, kind="ExternalInput")
    t_ftw = nc.dram_tensor("ftw", [ATOM, D], BF, kind="ExternalInput")
    t_bias = nc.dram_tensor("bias", [128, NCOL], F32, kind="ExternalInput")
    t_w = {}
    for l in range(L):
        for nm in ("saq", "sak", "sav", "sao", "eaq", "eak", "eav", "eao"):
            t_w[nm, l] = nc.dram_tensor(f"{nm}{l}", [D, D], BF, kind="ExternalInput")
        t_w["w1", l] = nc.dram_tensor(f"w1_{l}", [D, PF], BF, kind="ExternalInput")
        t_w["w2", l] = nc.dram_tensor(f"w2_{l}", [PF, D], BF, kind="ExternalInput")
    t_fc1 = nc.dram_tensor("fc1", [D, 256], BF, kind="ExternalInput")
    t_fc2 = nc.dram_tensor("fc2", [256, 2], BF, kind="ExternalInput")
    t_out = nc.dram_tensor("out", [1, 2], F32, kind="ExternalOutput")

    rg = [list(range(C))]

    with tile.TileContext(nc) as tc:
        with (
            tc.tile_pool(name="dram", bufs=1, space="DRAM") as dram,
            tc.tile_pool(name="const", bufs=1) as cons,
            tc.tile_pool(name="state", bufs=1) as st,
            tc.tile_pool(name="wts", bufs=2) as wp,
            tc.tile_pool(name="wkv", bufs=1) as wkv,
            tc.tile_pool(name="wff", bufs=1) as wff,
            tc.tile_pool(name="kv", bufs=1) as kvp,
            tc.tile_pool(name="work", bufs=2) as wk,
            tc.tile_pool(name="small", bufs=2) as sm,
            tc.tile_pool(name="psS", bufs=3, space="PSUM") as psS,
            tc.tile_pool(name="psO", bufs=2, space="PSUM") as psO,
            tc.tile_pool(name="psP", bufs=2, space="PSUM") as psP,
            tc.tile_pool(name="psL", bufs=1, space="PSUM") as psL,
        ):
            # ---------- constants ----------
            bias_sb = cons.tile([128, NCOL], F32, tag="bias")
            nc.sync.dma_start(bias_sb[:], t_bias[:])
            ones_sb = cons.tile([128, 1], F32, tag="ones")
            nc.gpsimd.memset(ones_sb[:], 1.0)
            eps_sb = cons.tile([1, 1], F32, tag="eps")
            nc.gpsimd.memset(eps_sb[:], EPS)
            wrhs_sb = cons.tile([128, TC], F32, tag="wrhs")
            nc.gpsimd.memset(wrhs_sb[:], 0.5)
            ones_bf = cons.tile([128, 1], BF, tag="ones_bf")
            nc.gpsimd.memset(ones_bf[:], 1.0)
            ftw_sb = cons.tile([ATOM, D], BF, tag="ftw")
            nc.sync.dma_start(ftw_sb[:], t_ftw[:])
            trgT_sb = cons.tile([ATOM, TC], BF, tag="trgT")
            nc.sync.dma_start(trgT_sb[:], t_trgT[:])
            srcT_sb = cons.tile([128, NC4, TC], BF, tag="srcT")
            nc.sync.dma_start(srcT_sb[:], t_srcT.ap().rearrange("(c p) t -> p c t", p=128))
            fc1_sb = cons.tile([128, NC4, 256], BF, tag="fc1")
            nc.sync.dma_start(fc1_sb[:], t_fc1.ap().rearrange("(c p) o -> p c o", p=128))
            fc2_sb = cons.tile([128, 2, 2], BF, tag="fc2")
            nc.sync.dma_start(fc2_sb[:], t_fc2.ap().rearrange("(c p) o -> p c o", p=128))

            # ---------- persistent state ----------
            xTf = st.tile([128, NC4, TC], F32, tag="xTf")     # x transposed, f32
            xTb = st.tile([128, NC4, TC], BF, tag="xTb")      # bf16 copy
            sq = st.tile([128, NC4, TC], F32, tag="sq")       # squares scratch

            def bcolap(col):
                return bias_sb[:, col:col + 1]

            def hrows(tl, h):
                """head h rows of a feat-packed [128, NC4, X] tile -> [64, X]."""
                o = 64 * (h % 2)
                return tl[o:o + 64, h // 2, :]

            def warm(n, anchor=None):
                # PE matmuls that keep the HAM clock-gate open while the
                # tensor engine would otherwise idle (AG waits). `anchor`
                # (a bf16 AP) sequences them after a producer so the
                # scheduler can't hoist them out of the idle window.
                for _ in range(n):
                    ps = psS.tile([1, TC], F32, tag="scoresT")
                    if anchor is None:
                        nc.tensor.matmul(ps[:], lhsT=ones_sb[:], rhs=wrhs_sb[:],
                                         start=True, stop=True)
                    else:
                        nc.tensor.matmul(ps[:], lhsT=ones_bf[:], rhs=anchor,
                                         start=True, stop=True)

            def load_w(tag, dram_t, nchunk, width, pool=None):
                w = (pool or wp).tile([128, nchunk, width], BF, tag=tag)
                nc.sync.dma_start(
                    w[:], dram_t.ap().rearrange("(c p) o -> p c o", p=128))
                return w

            def proj_T(out_sb, w_sb, rhs_sb, nch_in, m_tiles, bias_col=None,
                       func=ACTF.Identity, out2_sb=None):
                """out[feat,tok] (+bias, +func) = W^T @ rhs; evac on ScalarE."""
                for m in range(m_tiles):
                    ps = psP.tile([128, 512], F32, tag="psp")
                    for c in range(nch_in):
                        nc.tensor.matmul(
                            ps[:, 0:TC],
                            lhsT=w_sb[:, c, 128 * m:128 * m + 128],
                            rhs=rhs_sb[:, c, :],
                            start=(c == 0), stop=(c == nch_in - 1))
                    b = bcolap(bias_col + m) if bias_col is not None else 0.0
                    f = func if bias_col is not None else (
                        ACTF.Copy if func == ACTF.Identity else func)
                    nc.scalar.activation(out_sb[:, m, :], ps[:, 0:TC], f, bias=b)
                    if out2_sb is not None:
                        nc.scalar.activation(out2_sb[:, m, :], ps[:, 0:TC], ACTF.Copy)

            def v_natural(out_sb, wv_sb, rhsT_sb):
                """v[tok, feat] = x @ Wv (no bias; bv folded into bo on host)."""
                for tchunk in range(2):
                    ps = psP.tile([128, 512], F32, tag="psp")
                    for c in range(NC4):
                        nc.tensor.matmul(
                            ps[:],
                            lhsT=rhsT_sb[:, c, 128 * tchunk:128 * tchunk + 128],
                            rhs=wv_sb[:, c, :],
                            start=(c == 0), stop=(c == NC4 - 1))
                    nc.scalar.activation(out_sb[:, tchunk, :], ps[:], ACTF.Copy)

            def layer_norm(gcol, bcol):
                """resid[128,NC4,TC] f32 -> xTf, xTb (feat-dim LN in T layout)."""
                for m in range(NC4):
                    nc.vector.tensor_tensor(sq[:, m, :], resid[:, m, :],
                                            resid[:, m, :], op=ALU.mult)
                psl = psL.tile([33, TC], F32, tag="psl")
                for c in range(NC4):
                    nc.tensor.matmul(psl[0:1, :], lhsT=ones_sb[:], rhs=resid[:, c, :],
                                     start=(c == 0), stop=(c == NC4 - 1))
                for c in range(NC4):
                    nc.tensor.matmul(psl[32:33, :], lhsT=ones_sb[:], rhs=sq[:, c, :],
                                     start=(c == 0), stop=(c == NC4 - 1))
                mn = sm.tile([1, TC], F32, tag="mn")
                nc.scalar.mul(mn[:], psl[0:1, :], 1.0 / D)
                m2 = sm.tile([1, TC], F32, tag="m2")
                nc.vector.tensor_tensor(m2[:], mn[:], mn[:], op=ALU.mult)
                ve = sm.tile([1, TC], F32, tag="ve")
                nc.vector.scalar_tensor_tensor(ve[:], psl[32:33, :], 1.0 / D, m2[:],
                                               op0=ALU.mult, op1=ALU.subtract)
                vs = sm.tile([1, TC], F32, tag="vs")
                nc.scalar.activation(vs[:], ve[:], ACTF.Sqrt, bias=eps_sb[:])
                rs = sm.tile([1, TC], F32, tag="rs")
                nc.vector.reciprocal(rs[:], vs[:])
                mb = sm.tile([128, TC], F32, tag="mb")
                nc.gpsimd.partition_broadcast(mb[:], mn[:])
                rb = sm.tile([128, TC], F32, tag="rsb")
                nc.gpsimd.partition_broadcast(rb[:], rs[:])
                for m in range(NC4):
                    t1 = sm.tile([128, TC], F32, tag="t1")
                    nc.vector.tensor_tensor(t1[:], resid[:, m, :], mb[:], op=ALU.subtract)
                    t2 = sm.tile([128, TC], F32, tag="t2")
                    nc.vector.tensor_tensor(t2[:], t1[:], rb[:], op=ALU.mult)
                    nc.vector.tensor_scalar(xTf[:, m, :], t2[:],
                                            bcolap(gcol + m), bcolap(bcol + m),
                                            op0=ALU.mult, op1=ALU.add)
                    nc.scalar.activation(xTb[:, m, :], xTf[:, m, :], ACTF.Copy)

            def load_kv_from_ag(ag_out, KT, Vg):
                """ag_out [C*1024, TC] bf16 (per rank: kT [512,TC] then v flat)."""
                # KT [128, NC4, T]: per head-pair hp, gather all ranks
                src_k = ag_out[:].rearrange("(r x p) t -> x p r t", r=C, p=128)
                for hp in range(NC4):
                    nc.sync.dma_start(
                        KT[:, hp, :].rearrange("p (r t) -> p r t", t=TC),
                        src_k[hp])
                # Vg [128, NKT, 8*65+:] aug layout; v_r rows [1024r+512:1024r+1024]
                vsrc = ag_out[:].rearrange("(r v q) t -> r v q t", r=C, v=2)[:, 1, :, :]
                # vsrc: [C, 512, TC] rows of the v half; flat v[t, f] lives at
                # (row=2t+a, col=b) with f = 256a + b
                for r in range(C):
                    sr = vsrc[r].rearrange("(tc p a) b -> tc p (a b)", tc=2, a=2)
                    for tcn in range(2):
                        d = Vg[:, 2 * r + tcn, :].rearrange(
                            "p (h e) -> p h e", e=65)[:, :, 0:64]
                        nc.sync.dma_start(
                            d, sr[tcn].rearrange("p (h e) -> p h e", e=64))
                # ones columns for the softmax denominator
                nc.gpsimd.memset(
                    Vg[:].rearrange("p k (h e) -> p k h e", e=65)[:, :, :, 64:65], 1.0)

            def load_kT(ago_k, KT):
                src_k = ago_k[:].rearrange("(r x p) t -> x p r t", r=C, p=128)
                for hp in range(NC4):
                    nc.sync.dma_start(
                        KT[:, hp, :].rearrange("p (r t) -> p r t", t=TC),
                        src_k[hp])

            def load_V(ago_v, Vg):
                vsrc = ago_v[:].rearrange("(r q) t -> r q t", r=C, q=512)
                for r in range(C):
                    sr = vsrc[r].rearrange("(tc p a) b -> tc p (a b)", tc=2, a=2)
                    for tcn in range(2):
                        d = Vg[:, 2 * r + tcn, :].rearrange(
                            "p (h e) -> p h e", e=65)[:, :, 0:64]
                        nc.sync.dma_start(
                            d, sr[tcn].rearrange("p (h e) -> p h e", e=64))
                nc.gpsimd.memset(
                    Vg[:].rearrange("p k (h e) -> p k h e", e=65)[:, :, :, 64:65], 1.0)

            def store_kv_to_ag(ag_in, kT_loc, v_loc):
                nc.sync.dma_start(
                    ag_in[0:512, :].rearrange("(c p) t -> p c t", p=128), kT_loc[:])
                nc.sync.dma_start(
                    ag_in[512:1024, :].rearrange("(tc p a) b -> p tc (a b)",
                                                 tc=2, p=128, a=2), v_loc[:])

            def attention(KT, Vg, wq_sb, wo_sb, bq_col, bo_col):
                proj_T(qT, wq_sb, xTb, NC4, NC4, bias_col=bq_col)
                for h in range(H):
                    expT = wk.tile([128, NKT, TC], BF, tag="expT")
                    for kt in range(NKT):
                        ps = psS.tile([128, TC], F32, tag="scoresT")
                        nc.tensor.matmul(
                            ps[:],
                            lhsT=hrows(KT, h)[:, 128 * kt:128 * kt + 128],
                            rhs=hrows(qT, h),
                            start=True, stop=True)
                        nc.scalar.activation(expT[:, kt, :], ps[:], ACTF.Exp,
                                             scale=float(1.0 / np.sqrt(HD)))
                    pso = psO.tile([65, TC], F32, tag="psoT")
                    for kt in range(NKT):
                        nc.tensor.matmul(
                            pso[:],
                            lhsT=Vg[:, kt, 65 * h:65 * h + 65],
                            rhs=expT[:, kt, :],
                            start=(kt == 0), stop=(kt == NKT - 1))
                    den = sm.tile([1, TC], F32, tag="den")
                    nc.vector.reciprocal(den[:], pso[64:65, :])
                    rb = sm.tile([64, TC], F32, tag="rbh")
                    nc.gpsimd.partition_broadcast(rb[:], den[:])
                    nc.vector.tensor_tensor(hrows(oT, h), pso[0:64, :], rb[:],
                                            op=ALU.mult)
                # attn out projection + bias + residual -> resid (f32)
                for m in range(NC4):
                    ps = psP.tile([128, 512], F32, tag="psp")
                    for c in range(NC4):
                        nc.tensor.matmul(ps[:, 0:TC],
                                         lhsT=wo_sb[:, c, 128 * m:128 * m + 128],
                                         rhs=oT[:, c, :],
                                         start=(c == 0), stop=(c == NC4 - 1))
                    nc.vector.scalar_tensor_tensor(
                        resid[:, m, :], ps[:, 0:TC], bcolap(bo_col + m),
                        xTf[:, m, :], op0=ALU.add, op1=ALU.add)

            # ================= program =================
            # ft projection: xT = ftw^T @ trgT + ft_b
            resid = st.tile([128, NC4, TC], F32, tag="resid")
            qT = st.tile([128, NC4, TC], BF, tag="qT")
            oT = st.tile([128, NC4, TC], BF, tag="oT")
            ffT = st.tile([128, NPF, TC], BF, tag="ffT")
            kT_loc = st.tile([128, NC4, TC], BF, tag="kT_loc")
            v_loc = st.tile([128, 2, D], BF, tag="v_loc")

            warm(40)
            for m in range(NC4):
                ps = psP.tile([128, 512], F32, tag="psp")
                nc.tensor.matmul(ps[:, 0:TC],
                                 lhsT=ftw_sb[:, 128 * m:128 * m + 128],
                                 rhs=trgT_sb[:], start=True, stop=True)
                nc.scalar.activation(xTf[:, m, :], ps[:, 0:TC], ACTF.Identity,
                                     bias=bcolap(FT_B + m))
                nc.scalar.activation(xTb[:, m, :], ps[:, 0:TC], ACTF.Identity,
                                     bias=bcolap(FT_B + m))

            def sa_kv_gather(l):
                """project this core's K/V for layer l and issue split AGs."""
                kk = load_w("wk", t_w["sak", l], NC4, D, pool=wkv)
                vv = load_w("wv", t_w["sav", l], NC4, D, pool=wkv)
                proj_T(kT_loc, kk, xTb, NC4, NC4)
                v_natural(v_loc, vv, xTb)
                agi_k = dram.tile([512, TC], BF, tag=f"sa_agik{l}")
                ago_k = dram.tile([C * 512, TC], BF, tag=f"sa_agok{l}")
                agi_v = dram.tile([512, TC], BF, tag=f"sa_agiv{l}")
                ago_v = dram.tile([C * 512, TC], BF, tag=f"sa_agov{l}")
                nc.sync.dma_start(
                    agi_k[:].rearrange("(c p) t -> p c t", p=128), kT_loc[:])
                nc.sync.dma_start(
                    agi_v[:].rearrange("(tc p a) b -> p tc (a b)",
                                       tc=2, p=128, a=2), v_loc[:])
                nc.gpsimd.collective_compute(
                    "AllGather", ALU.bypass, replica_groups=rg,
                    ins=[agi_k[:]], outs=[ago_k[:]])
                nc.gpsimd.collective_compute(
                    "AllGather", ALU.bypass, replica_groups=rg,
                    ins=[agi_v[:]], outs=[ago_v[:]])
                return ago_k, ago_v

            # --- SA K/V of layer 0 first: its gather gates layer 0 ---
            wq_sa = load_w("wq", t_w["saq", 0], NC4, D)
            wo_sa = load_w("wo", t_w["sao", 0], NC4, D)
            sa_ag = sa_kv_gather(0)

            # --- EA K/V precompute (src-derived, all layers); the PE work
            # here overlaps the layer-0 SA gather ---
            ea_ag_out = []
            for l in range(L):
                wk_sb = load_w("wk", t_w["eak", l], NC4, D, pool=wkv)
                wv_sb = load_w("wv", t_w["eav", l], NC4, D, pool=wkv)
                proj_T(kT_loc, wk_sb, srcT_sb, NC4, NC4)
                v_natural(v_loc, wv_sb, srcT_sb)
                agi = dram.tile([1024, TC], BF, tag=f"ea_agi{l}")
                ago = dram.tile([C * 1024, TC], BF, tag=f"ea_ago{l}")
                store_kv_to_ag(agi, kT_loc, v_loc)
                nc.gpsimd.collective_compute(
                    "AllGather", ALU.bypass, replica_groups=rg,
                    ins=[agi[:]], outs=[ago[:]])
                ea_ag_out.append((agi, ago))

            KT_sa = kvp.tile([128, NC4, T], BF, tag="KT_sa")
            V_sa = kvp.tile([128, NKT, 8 * 65], BF, tag="V_sa")
            KT_ea = kvp.tile([128, NC4, T], BF, tag="KT_ea")
            V_ea = kvp.tile([128, NKT, 8 * 65], BF, tag="V_ea")

            for l in range(L):
                # ---- self attention ----
                if l > 0:
                    wq_sa = load_w("wq", t_w["saq", l], NC4, D)
                    wo_sa = load_w("wo", t_w["sao", l], NC4, D)
                warm(110, anchor=kT_loc[:, 0, :])
                load_kT(sa_ag[0], KT_sa)
                load_V(sa_ag[1], V_sa)
                attention(KT_sa, V_sa, wq_sa, wo_sa,
                          _bcol(l, SA_BQ), _bcol(l, SA_BO))
                layer_norm(_bcol(l, LNG), _bcol(l, LNB))

                # ---- cross attention ----
                wq_ea = load_w("wq", t_w["eaq", l], NC4, D)
                wo_ea = load_w("wo", t_w["eao", l], NC4, D)
                load_kv_from_ag(ea_ag_out[l][1], KT_ea, V_ea)
                attention(KT_ea, V_ea, wq_ea, wo_ea,
                          _bcol(l, EA_BQ), _bcol(l, EA_BO))
                layer_norm(_bcol(l, LNG), _bcol(l, LNB))

                # ---- next layer's SA K/V + gather (needs post-FFN x, so after FFN) ----
                # ---- FFN ----
                w1_sb = wff.tile([128, NC4, PF], BF, tag="w1")
                nc.sync.dma_start(w1_sb[:],
                                  t_w["w1", l].ap().rearrange("(c p) o -> p c o", p=128))
                w2_sb = wff.tile([128, NPF, D], BF, tag="w2")
                nc.sync.dma_start(w2_sb[:],
                                  t_w["w2", l].ap().rearrange("(c p) o -> p c o", p=128))
                for m in range(NPF):
                    ps = psP.tile([128, 512], F32, tag="psp")
                    for c in range(NC4):
                        nc.tensor.matmul(ps[:, 0:TC],
                                         lhsT=w1_sb[:, c, 128 * m:128 * m + 128],
                                         rhs=xTb[:, c, :],
                                         start=(c == 0), stop=(c == NC4 - 1))
                    nc.scalar.activation(ffT[:, m, :], ps[:, 0:TC], ACTF.Relu,
                                         bias=bcolap(_bcol(l, B1) + m))
                for m in range(NC4):
                    ps = psP.tile([128, 512], F32, tag="psp")
                    for c in range(NPF):
                        nc.tensor.matmul(ps[:, 0:TC],
                                         lhsT=w2_sb[:, c, 128 * m:128 * m + 128],
                                         rhs=ffT[:, c, :],
                                         start=(c == 0), stop=(c == NPF - 1))
                    nc.vector.scalar_tensor_tensor(
                        resid[:, m, :], ps[:, 0:TC], bcolap(_bcol(l, B2) + m),
                        xTf[:, m, :], op0=ALU.add, op1=ALU.add)
                layer_norm(_bcol(l, LNG), _bcol(l, LNB))

                if l + 1 < L:
                    sa_ag = sa_kv_gather(l + 1)

            # ---- pooling: softmax over token norms, then weighted sum ----
            for m in range(NC4):
                nc.vector.tensor_tensor(sq[:, m, :], xTf[:, m, :], xTf[:, m, :],
                                        op=ALU.mult)
            psl = psL.tile([33, TC], F32, tag="psl")
            for c in range(NC4):
                nc.tensor.matmul(psl[0:1, :], lhsT=ones_sb[:], rhs=sq[:, c, :],
                                 start=(c == 0), stop=(c == NC4 - 1))
            nrm = sm.tile([1, TC], F32, tag="nrm")
            nc.scalar.activation(nrm[:], psl[0:1, :], ACTF.Sqrt)
            ew = sm.tile([1, TC], F32, tag="ew")
            nc.scalar.activation(ew[:], nrm[:], ACTF.Exp)
            denl = sm.tile([1, 1], F32, tag="denl")
            nc.vector.reduce_sum(denl[:], ew[:], axis=AX.X)
            ewb = sm.tile([128, TC], F32, tag="ewb")
            nc.gpsimd.partition_broadcast(ewb[:], ew[:])
            ws = sm.tile([128, NC4 + 1], F32, tag="ws")
            for m in range(NC4):
                t1 = sm.tile([128, TC], F32, tag="t1")
                nc.vector.tensor_tensor(t1[:], xTf[:, m, :], ewb[:], op=ALU.mult)
                nc.vector.reduce_sum(ws[:, m:m + 1], t1[:], axis=AX.X)
            nc.vector.tensor_copy(ws[0:1, NC4:NC4 + 1], denl[:])

            ar_in = dram.tile([513, 1], F32, tag="ar_in")
            ar_out = dram.tile([513, 1], F32, tag="ar_out")
            nc.sync.dma_start(
                ar_in[0:512, :].rearrange("(c p) o -> p (c o)", p=128),
                ws[:, 0:NC4])
            nc.sync.dma_start(ar_in[512:513, :], ws[0:1, NC4:NC4 + 1])
            nc.gpsimd.collective_compute("AllReduce", ALU.add, replica_groups=rg,
                                         ins=[ar_in[:]], outs=[ar_out[:]])

            wsg = sm.tile([128, NC4], F32, tag="wsg")
            nc.sync.dma_start(
                wsg[:], ar_out[0:512, :].rearrange("(c p) o -> p (c o)", p=128))
            deng = sm.tile([1, 1], F32, tag="deng")
            nc.sync.dma_start(deng[:], ar_out[512:513, :])
            rd = sm.tile([1, 1], F32, tag="rd")
            nc.vector.reciprocal(rd[:], deng[:])
            rdb = sm.tile([128, 1], F32, tag="rdb")
            nc.gpsimd.partition_broadcast(rdb[:], rd[:])
            pooledT = sm.tile([128, NC4], BF, tag="pooledT")
            nc.vector.tensor_scalar_mul(pooledT[:], wsg[:], rdb[:])

            h1T = sm.tile([128, 2, 1], BF, tag="h1T")
            for m in range(2):
                ps = psP.tile([128, 512], F32, tag="psp")
                for c in range(NC4):
                    nc.tensor.matmul(ps[:, 0:1],
                                     lhsT=fc1_sb[:, c, 128 * m:128 * m + 128],
                                     rhs=pooledT[:, c:c + 1],
                                     start=(c == 0), stop=(c == NC4 - 1))
                nc.scalar.activation(h1T[:, m, :], ps[:, 0:1], ACTF.Relu,
                                     bias=bcolap(FC1_B + m))
            ps2 = psP.tile([128, 512], F32, tag="psp")
            for c in range(2):
                nc.tensor.matmul(ps2[0:2, 0:1], lhsT=fc2_sb[:, c, :],
                                 rhs=h1T[:, c, :],
                                 start=(c == 0), stop=(c == 1))
            lab = sm.tile([2, 1], F32, tag="lab")
            nc.scalar.activation(lab[:], ps2[0:2, 0:1], ACTF.Identity,
                                 bias=bias_sb[0:2, FC2_B:FC2_B + 1])
            nc.sync.dma_start(t_out.ap().rearrange("a b -> b a"), lab[:])

    nc.compile()
    return nc


_PROGRAM = None


def _get_program():
    global _PROGRAM
    if _PROGRAM is None:
        _PROGRAM = build_program()
    return _PROGRAM


def _host_inputs(inputs):
    f = {k: np.asarray(v, np.float32) for k, v in inputs.items()}

    def bf(x):
        return np.ascontiguousarray(np.asarray(x, np.float32).astype(BF16))

    bias = np.zeros((128, NCOL), np.float32)

    def put(col, vec):
        v = np.asarray(vec, np.float32).reshape(-1)
        for c in range(len(v) // 128):
            bias[:, col + c] = v[128 * c:128 * c + 128]

    put(FT_B, f['ft_b'])
    for l in range(L):
        put(_bcol(l, SA_BQ), f['sa_bq'][l])
        put(_bcol(l, SA_BO), f['sa_bv'][l] @ f['sa_wo'][l] + f['sa_bo'][l])
        put(_bcol(l, EA_BQ), f['ea_bq'][l])
        put(_bcol(l, EA_BO), f['ea_bv'][l] @ f['ea_wo'][l] + f['ea_bo'][l])
        put(_bcol(l, B1), f['pf_b1'][l])
        put(_bcol(l, B2), f['pf_b2'][l])
        put(_bcol(l, LNG), f['ln_g'][l])
        put(_bcol(l, LNB), f['ln_b'][l])
    put(FC1_B, f['fc1_b'])
    bias[0:2, FC2_B] = f['fc2_b']

    shared = {'ftw': bf(f['ft_w']), 'bias': bias,
              'fc1': bf(f['fc1_w']), 'fc2': bf(f['fc2_w'])}
    for l in range(L):
        shared[f'saq{l}'] = bf(f['sa_wq'][l])
        shared[f'sak{l}'] = bf(f['sa_wk'][l])
        shared[f'sav{l}'] = bf(f['sa_wv'][l])
        shared[f'sao{l}'] = bf(f['sa_wo'][l])
        shared[f'eaq{l}'] = bf(f['ea_wq'][l])
        shared[f'eak{l}'] = bf(f['ea_wk'][l])
        shared[f'eav{l}'] = bf(f['ea_wv'][l])
        shared[f'eao{l}'] = bf(f['ea_wo'][l])
        shared[f'w1_{l}'] = bf(f['pf_w1'][l])
        shared[f'w2_{l}'] = bf(f['pf_w2'][l])

    in_maps = []
    for i in range(C):
        sl = slice(TC * i, TC * (i + 1))
        m = dict(shared)
        m['trgT'] = bf(f['trg'][0, sl, :].T)
        m['srcT'] = bf(f['src'][0, sl, :].T)
        in_maps.append(m)
    return in_maps


def kernel(**inputs):
    import os
    nc = _get_program()
    in_maps = _host_inputs(inputs)
    trace = bool(int(os.environ.get("KERNEL_TRACE", "0")))
    res = bass_utils.run_bass_kernel_spmd(
        nc, in_maps, core_ids=list(range(C)), trace=trace)
    if trace:
        kernel.last_exec_time_ns = res.exec_time_ns
    return np.asarray(res.results[0]["out"], np.float32)



# revision 7
# speedup vs baseline: 1.3384x; 1.3384x over previous
"""Trainium2 Bass kernel for nn_Decoder_5334349382400.

3-layer transformer decoder (self-attn + cross-attn + FFN + LN) with
norm-softmax pooling and a 2-class head, batch=1, seq 2048, hid 512.

Sharding: sequence-parallel over 8 NeuronCores (256 tokens/core).
 - All per-token work (projections, FFN, LN, softmax rows) is local.
 - Self-attention K/V are computed locally per-core and AllGathered
   (bf16, merged k+v) once per layer; cross-attention K/V depend only
   on `src`, so they are computed+gathered once for all 3 layers.
 - Final pooling uses a tiny AllReduce of [wsum(512) | denom(1)].

Layout: activations live transposed in SBUF as bf16, xT[feat(part),
tok(free)], packed [128, 4, 256] (feat chunk-major).  Head h of a
[128, 4, X] tile lives at partitions 64*(h%2).. of chunk h//2, so a
head PAIR (2j, 2j+1) occupies the full 128 partitions of chunk j and
scores matmuls of a pair run concurrently via row tile-positions
(0,0)/(64,0).  Scores for 2 kt-tiles x 2 heads accumulate into one
2-bank PSUM region [128,1024] and are exp'd by ONE ScalarE activation
(amortizes the ~350cyc/op overhead).  Softmax denominator rides as a
ones-column appended to V (65-row AV output); normalization happens on
the AV result via reciprocal_approx_fast + gpsimd row-broadcast.
LN avoids Sqrt (stays in the exp/ln table set): rstd = exp(-0.5*ln(v+eps)).
All PSUM evacuations run on VectorE with fused bias/relu/residual.
"""

import sys

sys.path.insert(0, "/opt/trn_rl_repo")

import numpy as np
import ml_dtypes

import concourse.bass as bass
import concourse.mybir as mybir
import concourse.tile as tile
from concourse import bacc, bass_utils

BF16 = ml_dtypes.bfloat16
F32 = mybir.dt.float32
BF = mybir.dt.bfloat16
AX = mybir.AxisListType
ALU = mybir.AluOpType
ACTF = mybir.ActivationFunctionType

C = 8          # cores
T = 2048       # tokens
TC = T // C    # tokens per core (256)
D = 512        # hidden
H = 8          # heads
HD = 64        # head dim
PF = 2048      # ffn dim
L = 3          # layers
ATOM = 64      # trg feature dim
NC4 = D // 128   # 4 feature chunks
NPF = PF // 128  # 16
NKT = T // 128   # 16 key tiles
EPS = 1e-5

# bias-pack column map
FT_B = 0
LBASE = 4
LSTRIDE = 44
SA_BQ, SA_BO, EA_BQ, EA_BO, B1, B2, LNG, LNB = 0, 4, 8, 12, 16, 32, 36, 40
FC1_B = LBASE + L * LSTRIDE          # 136
FC2_B = FC1_B + 2                    # 138
NCOL = FC2_B + 1                     # 139


def _bcol(l, off):
    return LBASE + l * LSTRIDE + off


def build_program():
    nc = bacc.Bacc("TRN2", target_bir_lowering=False, debug=False,
                   enable_asserts=True, num_devices=C)

    # ---- DRAM I/O ----
    t_trgT = nc.dram_tensor("trgT", [ATOM, TC], BF, kind="ExternalInput")
    t_srcT = nc.dram_tensor("srcT", [D, TC], BF, kind="ExternalInput")
    t_ftw = nc.dram_tensor("ftw", [ATOM, D], BF, kind="ExternalInput")
    t_bias = nc.dram_tensor("bias", [128, NCOL], F32, kind="ExternalInput")
    t_w = {}
    for l in range(L):
        for nm in ("saq", "sak", "sav", "sao", "eaq", "eak", "eav", "eao"):
            t_w[nm, l] = nc.dram_tensor(f"{nm}{l}", [D, D], BF, kind="ExternalInput")
        t_w["w1", l] = nc.dram_tensor(f"w1_{l}", [D, PF], BF, kind="ExternalInput")
        t_w["w2", l] = nc.dram_tensor(f"w2_{l}", [PF, D], BF, kind="ExternalInput")
    t_fc1 = nc.dram_tensor("fc1", [D, 256], BF, kind="ExternalInput")
    t_fc2 = nc.dram_tensor("fc2", [256, 2], BF, kind="ExternalInput")
    t_out = nc.dram_tensor("out", [1, 2], F32, kind="ExternalOutput")

    rg = [list(range(C))]

    with tile.TileContext(nc) as tc:
        with (
            tc.tile_pool(name="dram", bufs=1, space="DRAM") as dram,
            tc.tile_pool(name="dramS", bufs=1, space="DRAM") as dramS,
            tc.tile_pool(name="const", bufs=1) as cons,
            tc.tile_pool(name="state", bufs=1) as st,
            tc.tile_pool(name="wts", bufs=2) as wp,
            tc.tile_pool(name="wkv", bufs=1) as wkv,
            tc.tile_pool(name="wff", bufs=1) as wff,
            tc.tile_pool(name="kv", bufs=1) as kvp,
            tc.tile_pool(name="exppool", bufs=4) as xp,
            tc.tile_pool(name="small", bufs=2) as sm,
            tc.tile_pool(name="psS", bufs=2, space="PSUM") as psS,
            tc.tile_pool(name="psO", bufs=2, space="PSUM") as psO,
            tc.tile_pool(name="psP", bufs=2, space="PSUM") as psP,
        ):
            # ---------- constants ----------
            bias_sb = cons.tile([128, NCOL], F32, tag="bias")
            nc.sync.dma_start(bias_sb[:], t_bias[:])
            ones_bf = cons.tile([128, 1], BF, tag="ones_bf")
            nc.gpsimd.memset(ones_bf[:], 1.0)
            eps_sb = cons.tile([1, 1], F32, tag="eps")
            nc.gpsimd.memset(eps_sb[:], EPS)
            wrhs_bf = cons.tile([1, 256], BF, tag="wrhs")
            nc.gpsimd.memset(wrhs_bf[:], 0.5)
            ftw_sb = cons.tile([ATOM, D], BF, tag="ftw")
            nc.sync.dma_start(ftw_sb[:], t_ftw[:])
            trgT_sb = cons.tile([ATOM, TC], BF, tag="trgT")
            nc.sync.dma_start(trgT_sb[:], t_trgT[:])
            srcT_sb = cons.tile([128, NC4, TC], BF, tag="srcT")
            nc.sync.dma_start(srcT_sb[:], t_srcT.ap().rearrange("(c p) t -> p c t", p=128))
            fc1_sb = cons.tile([128, NC4, 256], BF, tag="fc1")
            nc.sync.dma_start(fc1_sb[:], t_fc1.ap().rearrange("(c p) o -> p c o", p=128))
            fc2_sb = cons.tile([128, 2, 2], BF, tag="fc2")
            nc.sync.dma_start(fc2_sb[:], t_fc2.ap().rearrange("(c p) o -> p c o", p=128))

            # ---------- persistent state (bf16) ----------
            xT = st.tile([128, NC4, TC], BF, tag="xT")         # LN output
            resid = st.tile([128, NC4, TC], BF, tag="resid")   # pre-LN sum
            sq = st.tile([128, NC4, TC], BF, tag="sq")         # squares
            qT = st.tile([128, NC4, TC], BF, tag="qT")
            oT = st.tile([128, NC4, TC], BF, tag="oT")
            ffT = st.tile([128, NPF, TC], BF, tag="ffT")
            kT_loc = st.tile([128, NC4, TC], BF, tag="kT_loc")
            v_loc = st.tile([128, 2, D], BF, tag="v_loc")

            def bcolap(col):
                return bias_sb[:, col:col + 1]

            def hrows(tl, h):
                """head h rows of a feat-packed [128, NC4, X] tile -> [64, X]."""
                o = 64 * (h % 2)
                return tl[o:o + 64, h // 2, :]

            def warm(n, anchor=None):
                # PE matmuls that keep the HAM clock-gate open while the
                # tensor engine would otherwise idle (AG waits). `anchor`
                # (a bf16 AP) sequences them after a producer so the
                # scheduler can't hoist them out of the idle window.
                for _ in range(n):
                    ps = psS.tile([128, 4, 256], F32, tag="scores")
                    nc.tensor.matmul(ps[0:1, 0, :], lhsT=ones_bf[0:1, :],
                                     rhs=(wrhs_bf[:] if anchor is None else anchor),
                                     start=True, stop=True)

            def load_w(tag, dram_t, nchunk, width, pool=None):
                w = (pool or wp).tile([128, nchunk, width], BF, tag=tag)
                nc.sync.dma_start(
                    w[:], dram_t.ap().rearrange("(c p) o -> p c o", p=128))
                return w

            def proj_T(out_sb, w_sb, rhs_sb, nch_in, m_tiles, bias_col=None,
                       relu=False):
                """out[feat,tok] bf16 = W^T @ rhs (+bias) (+relu); DVE evac."""
                for m in range(m_tiles):
                    ps = psP.tile([128, 512], F32, tag="psp")
                    for c in range(nch_in):
                        nc.tensor.matmul(
                            ps[:, 0:TC],
                            lhsT=w_sb[:, c, 128 * m:128 * m + 128],
                            rhs=rhs_sb[:, c, :],
                            start=(c == 0), stop=(c == nch_in - 1))
                    b = bcolap(bias_col + m) if bias_col is not None else 0.0
                    if relu:
                        nc.vector.tensor_scalar(out_sb[:, m, :], ps[:, 0:TC],
                                                b, 0.0, op0=ALU.add, op1=ALU.max)
                    else:
                        nc.vector.tensor_scalar(out_sb[:, m, :], ps[:, 0:TC],
                                                b, None, op0=ALU.add)

            def v_natural(out_sb, wv_sb, rhsT_sb):
                """v[tok, feat] = x @ Wv (no bias; bv folded into bo on host)."""
                for tchunk in range(2):
                    ps = psP.tile([128, 512], F32, tag="psp")
                    for c in range(NC4):
                        nc.tensor.matmul(
                            ps[:],
                            lhsT=rhsT_sb[:, c, 128 * tchunk:128 * tchunk + 128],
                            rhs=wv_sb[:, c, :],
                            start=(c == 0), stop=(c == NC4 - 1))
                    nc.vector.tensor_copy(out_sb[:, tchunk, :], ps[:])

            def layer_norm(gcol, bcol):
                """resid[128,NC4,TC] bf16 -> xT bf16 (feat-dim LN, T layout).

                rstd = exp(-0.5*ln(var+eps)) keeps ScalarE in the exp/ln
                table set (no Sqrt table switch)."""
                for m in range(NC4):
                    nc.vector.tensor_tensor(sq[:, m, :], resid[:, m, :],
                                            resid[:, m, :], op=ALU.mult)
                psl = psP.tile([128, 512], F32, tag="psp")
                for c in range(NC4):
                    nc.tensor.matmul(psl[0:1, 0:TC], lhsT=ones_bf[:],
                                     rhs=resid[:, c, :],
                                     start=(c == 0), stop=(c == NC4 - 1))
                for c in range(NC4):
                    nc.tensor.matmul(psl[32:33, 0:TC], lhsT=ones_bf[:],
                                     rhs=sq[:, c, :],
                                     start=(c == 0), stop=(c == NC4 - 1))
                mn = sm.tile([1, TC], BF, tag="mn")
                nc.vector.tensor_scalar(mn[:], psl[0:1, 0:TC], 1.0 / D, None,
                                        op0=ALU.mult)
                mb = sm.tile([128, TC], BF, tag="mb")
                nc.gpsimd.partition_broadcast(mb[:], mn[:])
                m2 = sm.tile([1, TC], F32, tag="m2")
                nc.vector.tensor_tensor(m2[:], mn[:], mn[:], op=ALU.mult)
                ve = sm.tile([1, TC], F32, tag="ve")
                nc.vector.scalar_tensor_tensor(ve[:], psl[32:33, 0:TC], 1.0 / D,
                                               m2[:], op0=ALU.mult,
                                               op1=ALU.subtract)
                lnv = sm.tile([1, TC], F32, tag="lnv")
                nc.scalar.activation(lnv[:], ve[:], ACTF.Ln, bias=eps_sb[:])
                rs = sm.tile([1, TC], BF, tag="rs")
                nc.scalar.activation(rs[:], lnv[:], ACTF.Exp, scale=-0.5)
                rb = sm.tile([128, TC], BF, tag="rsb")
                nc.gpsimd.partition_broadcast(rb[:], rs[:])
                for m in range(NC4):
                    t1 = sm.tile([128, TC], BF, tag="t1")
                    nc.vector.tensor_tensor(t1[:], resid[:, m, :], mb[:],
                                            op=ALU.subtract)
                    t2 = sm.tile([128, TC], BF, tag="t2")
                    nc.vector.tensor_tensor(t2[:], t1[:], rb[:], op=ALU.mult)
                    nc.vector.tensor_scalar(xT[:, m, :], t2[:],
                                            bcolap(gcol + m), bcolap(bcol + m),
                                            op0=ALU.mult, op1=ALU.add)

            def load_kv_from_ag(ago, KT, Vg):
                """ago [C*1024, TC] bf16 (per rank: kT [512,TC] then v flat)."""
                src_k = ago[:].rearrange("(r x p) t -> x p r t", r=C, p=128)
                for hp in range(NC4):
                    nc.sync.dma_start(
                        KT[:, hp, :].rearrange("p (r t) -> p r t", t=TC),
                        src_k[hp])
                vsrc = ago[:].rearrange("(r v q) t -> r v q t", r=C, v=2)[:, 1, :, :]
                for r in range(C):
                    sr = vsrc[r].rearrange("(tc p a) b -> tc p (a b)", tc=2, a=2)
                    for tcn in range(2):
                        d = Vg[:, 2 * r + tcn, :].rearrange(
                            "p (h e) -> p h e", e=65)[:, :, 0:64]
                        nc.sync.dma_start(
                            d, sr[tcn].rearrange("p (h e) -> p h e", e=64))
                nc.gpsimd.memset(
                    Vg[:].rearrange("p k (h e) -> p k h e", e=65)[:, :, :, 64:65], 1.0)

            def store_kv_to_ag(ag_in, kT_l, v_l):
                nc.sync.dma_start(
                    ag_in[0:512, :].rearrange("(c p) t -> p c t", p=128), kT_l[:])
                nc.sync.dma_start(
                    ag_in[512:1024, :].rearrange("(tc p a) b -> p tc (a b)",
                                                 tc=2, p=128, a=2), v_l[:])

            def attention(KT, Vg, wq_sb, wo_sb, bq_col, bo_col):
                proj_T(qT, wq_sb, xT, NC4, NC4, bias_col=bq_col)
                for j in range(4):              # head pair (A, B) = (2j, 2j+1)
                    A, B = 2 * j, 2 * j + 1
                    pso_a = psO.tile([65, TC], F32, tag="pso")
                    pso_b = psO.tile([65, TC], F32, tag="pso")
                    for rnd in range(8):        # kt block of 2 per round
                        k0 = 2 * rnd
                        ps = psS.tile([128, 4, 256], F32, tag="scores")
                        # scores: A/B interleaved -> row tile-positions
                        # (0,0)/(64,0) run concurrently on the PE array.
                        # psum columns: [A-kt0, A-kt1, B-kt0, B-kt1]
                        for dk in range(2):
                            for i, h in ((0, A), (2, B)):
                                nc.tensor.matmul(
                                    ps[:, i + dk, :],
                                    lhsT=hrows(KT, h)[:, 128 * (k0 + dk):
                                                      128 * (k0 + dk) + 128],
                                    rhs=hrows(qT, h),
                                    start=True, stop=True)
                        expT = xp.tile([128, 4, 256], BF, tag="expT")
                        nc.scalar.activation(expT[:], ps[:], ACTF.Exp,
                                             scale=float(1.0 / np.sqrt(HD)))
                        for dk in range(2):
                            for i, h, pso in ((0, A, pso_a), (2, B, pso_b)):
                                nc.tensor.matmul(
                                    pso[:],
                                    lhsT=Vg[:, k0 + dk, 65 * h:65 * h + 65],
                                    rhs=expT[:, i + dk, :],
                                    start=(rnd == 0 and dk == 0),
                                    stop=(rnd == 7 and dk == 1))
                    # evacuate: oT[h] = pso[0:64] * broadcast(1/den[h])
                    for h, pso in ((A, pso_a), (B, pso_b)):
                        rdf = sm.tile([1, TC], F32, tag="rdf")
                        nc.vector.reciprocal(rdf[:], pso[64:65, :])
                        rdb = sm.tile([64, TC], F32, tag="rdb")
                        nc.gpsimd.partition_broadcast(rdb[:], rdf[:])
                        nc.vector.tensor_tensor(hrows(oT, h), pso[0:64, :],
                                                rdb[:], op=ALU.mult)
                # attn out projection + bias + residual -> resid (bf16)
                for m in range(NC4):
                    ps = psP.tile([128, 512], F32, tag="psp")
                    for c in range(NC4):
                        nc.tensor.matmul(ps[:, 0:TC],
                                         lhsT=wo_sb[:, c, 128 * m:128 * m + 128],
                                         rhs=oT[:, c, :],
                                         start=(c == 0), stop=(c == NC4 - 1))
                    nc.vector.scalar_tensor_tensor(
                        resid[:, m, :], ps[:, 0:TC], bcolap(bo_col + m),
                        xT[:, m, :], op0=ALU.add, op1=ALU.add)

            # ================= program =================
            # ft projection: xT = ftw^T @ trgT + ft_b
            warm(20)
            for m in range(NC4):
                ps = psP.tile([128, 512], F32, tag="psp")
                nc.tensor.matmul(ps[:, 0:TC],
                                 lhsT=ftw_sb[:, 128 * m:128 * m + 128],
                                 rhs=trgT_sb[:], start=True, stop=True)
                nc.vector.tensor_scalar(xT[:, m, :], ps[:, 0:TC],
                                        bcolap(FT_B + m), None, op0=ALU.add)

            def sa_kv_gather(l):
                """project this core's K/V for layer l; one merged AG."""
                kk = load_w("wk", t_w["sak", l], NC4, D, pool=wkv)
                vv = load_w("wv", t_w["sav", l], NC4, D, pool=wkv)
                proj_T(kT_loc, kk, xT, NC4, NC4)
                v_natural(v_loc, vv, xT)
                agi = dram.tile([1024, TC], BF, tag=f"sa_agi{l}")
                ago = dramS.tile([C * 1024, TC], BF, tag=f"sa_ago{l}",
                                 addr_space="Shared")
                store_kv_to_ag(agi, kT_loc, v_loc)
                nc.gpsimd.collective_compute(
                    "AllGather", ALU.bypass, replica_groups=rg,
                    ins=[agi[:]], outs=[ago[:]])
                return ago

            # --- SA K/V of layer 0 first: its gather gates layer 0 ---
            wq_sa = load_w("wq", t_w["saq", 0], NC4, D)
            wo_sa = load_w("wo", t_w["sao", 0], NC4, D)
            sa_ago = sa_kv_gather(0)

            # --- EA K/V precompute (src-derived, all layers); the PE work
            # here overlaps the layer-0 SA gather ---
            ea_ago = []
            for l in range(L):
                wk_sb = load_w("wk", t_w["eak", l], NC4, D, pool=wkv)
                wv_sb = load_w("wv", t_w["eav", l], NC4, D, pool=wkv)
                proj_T(kT_loc, wk_sb, srcT_sb, NC4, NC4)
                v_natural(v_loc, wv_sb, srcT_sb)
                agi = dram.tile([1024, TC], BF, tag=f"ea_agi{l}")
                ago = dramS.tile([C * 1024, TC], BF, tag=f"ea_ago{l}",
                                 addr_space="Shared")
                store_kv_to_ag(agi, kT_loc, v_loc)
                nc.gpsimd.collective_compute(
                    "AllGather", ALU.bypass, replica_groups=rg,
                    ins=[agi[:]], outs=[ago[:]])
                ea_ago.append(ago)

            KT_sa = kvp.tile([128, NC4, T], BF, tag="KT_sa")
            V_sa = kvp.tile([128, NKT, 8 * 65], BF, tag="V_sa")
            KT_ea = kvp.tile([128, NC4, T], BF, tag="KT_ea")
            V_ea = kvp.tile([128, NKT, 8 * 65], BF, tag="V_ea")

            for l in range(L):
                # ---- self attention ----
                if l > 0:
                    wq_sa = load_w("wq", t_w["saq", l], NC4, D)
                    wo_sa = load_w("wo", t_w["sao", l], NC4, D)
                warm(55, anchor=kT_loc[0:1, 0, :])
                load_kv_from_ag(sa_ago, KT_sa, V_sa)
                attention(KT_sa, V_sa, wq_sa, wo_sa,
                          _bcol(l, SA_BQ), _bcol(l, SA_BO))
                layer_norm(_bcol(l, LNG), _bcol(l, LNB))

                # ---- cross attention ----
                wq_ea = load_w("wq", t_w["eaq", l], NC4, D)
                wo_ea = load_w("wo", t_w["eao", l], NC4, D)
                load_kv_from_ag(ea_ago[l], KT_ea, V_ea)
                attention(KT_ea, V_ea, wq_ea, wo_ea,
                          _bcol(l, EA_BQ), _bcol(l, EA_BO))
                layer_norm(_bcol(l, LNG), _bcol(l, LNB))

                # ---- FFN ----
                w1_sb = wff.tile([128, NC4, PF], BF, tag="w1")
                nc.sync.dma_start(w1_sb[:],
                                  t_w["w1", l].ap().rearrange("(c p) o -> p c o", p=128))
                w2_sb = wff.tile([128, NPF, D], BF, tag="w2")
                nc.sync.dma_start(w2_sb[:],
                                  t_w["w2", l].ap().rearrange("(c p) o -> p c o", p=128))
                proj_T(ffT, w1_sb, xT, NC4, NPF, bias_col=_bcol(l, B1),
                       relu=True)
                for m in range(NC4):
                    ps = psP.tile([128, 512], F32, tag="psp")
                    for c in range(NPF):
                        nc.tensor.matmul(ps[:, 0:TC],
                                         lhsT=w2_sb[:, c, 128 * m:128 * m + 128],
                                         rhs=ffT[:, c, :],
                                         start=(c == 0), stop=(c == NPF - 1))
                    nc.vector.scalar_tensor_tensor(
                        resid[:, m, :], ps[:, 0:TC], bcolap(_bcol(l, B2) + m),
                        xT[:, m, :], op0=ALU.add, op1=ALU.add)
                layer_norm(_bcol(l, LNG), _bcol(l, LNB))

                # ---- next layer's SA K/V + gather (needs post-FFN x) ----
                if l + 1 < L:
                    sa_ago = sa_kv_gather(l + 1)

            # ---- pooling: softmax over token norms, then weighted sum ----
            for m in range(NC4):
                nc.vector.tensor_tensor(sq[:, m, :], xT[:, m, :], xT[:, m, :],
                                        op=ALU.mult)
            psl = psP.tile([128, 512], F32, tag="psp")
            for c in range(NC4):
                nc.tensor.matmul(psl[0:1, 0:TC], lhsT=ones_bf[:],
                                 rhs=sq[:, c, :],
                                 start=(c == 0), stop=(c == NC4 - 1))
            lnn = sm.tile([1, TC], F32, tag="lnn")
            nc.scalar.activation(lnn[:], psl[0:1, 0:TC], ACTF.Ln)
            nrm = sm.tile([1, TC], F32, tag="nrm")
            nc.scalar.activation(nrm[:], lnn[:], ACTF.Exp, scale=0.5)
            ew = sm.tile([1, TC], BF, tag="ew")
            nc.scalar.activation(ew[:], nrm[:], ACTF.Exp)
            denl = sm.tile([1, 1], F32, tag="denl")
            nc.vector.reduce_sum(denl[:], ew[:], axis=AX.X)
            ewb = sm.tile([128, TC], BF, tag="ewb")
            nc.gpsimd.partition_broadcast(ewb[:], ew[:])
            ws = sm.tile([128, NC4 + 1], F32, tag="ws")
            for m in range(NC4):
                tp1 = sm.tile([128, TC], F32, tag="tp1")
                nc.vector.tensor_tensor(tp1[:], xT[:, m, :], ewb[:], op=ALU.mult)
                nc.vector.reduce_sum(ws[:, m:m + 1], tp1[:], axis=AX.X)
            nc.vector.tensor_copy(ws[0:1, NC4:NC4 + 1], denl[:])

            ar_in = dram.tile([513, 1], F32, tag="ar_in")
            ar_out = dramS.tile([513, 1], F32, tag="ar_out",
                                addr_space="Shared")
            nc.sync.dma_start(
                ar_in[0:512, :].rearrange("(c p) o -> p (c o)", p=128),
                ws[:, 0:NC4])
            nc.sync.dma_start(ar_in[512:513, :], ws[0:1, NC4:NC4 + 1])
            nc.gpsimd.collective_compute("AllReduce", ALU.add, replica_groups=rg,
                                         ins=[ar_in[:]], outs=[ar_out[:]])

            wsg = sm.tile([128, NC4], F32, tag="wsg")
            nc.sync.dma_start(
                wsg[:], ar_out[0:512, :].rearrange("(c p) o -> p (c o)", p=128))
            deng = sm.tile([1, 1], F32, tag="deng")
            nc.sync.dma_start(deng[:], ar_out[512:513, :])
            rd = sm.tile([1, 1], F32, tag="rd")
            nc.vector.reciprocal(rd[:], deng[:])
            rdb1 = sm.tile([128, 1], F32, tag="rdb1")
            nc.gpsimd.partition_broadcast(rdb1[:], rd[:])
            pooledT = sm.tile([128, NC4], BF, tag="pooledT")
            nc.vector.tensor_scalar_mul(pooledT[:], wsg[:], rdb1[:])

            h1T = sm.tile([128, 2, 1], BF, tag="h1T")
            for m in range(2):
                ps = psP.tile([128, 512], F32, tag="psp")
                for c in range(NC4):
                    nc.tensor.matmul(ps[:, 0:1],
                                     lhsT=fc1_sb[:, c, 128 * m:128 * m + 128],
                                     rhs=pooledT[:, c:c + 1],
                                     start=(c == 0), stop=(c == NC4 - 1))
                nc.vector.tensor_scalar(h1T[:, m, :], ps[:, 0:1],
                                        bcolap(FC1_B + m), 0.0,
                                        op0=ALU.add, op1=ALU.max)
            ps2 = psP.tile([128, 512], F32, tag="psp")
            for c in range(2):
                nc.tensor.matmul(ps2[0:2, 0:1], lhsT=fc2_sb[:, c, :],
                                 rhs=h1T[:, c, :],
                                 start=(c == 0), stop=(c == 1))
            lab = sm.tile([2, 1], F32, tag="lab")
            nc.vector.tensor_scalar(lab[:], ps2[0:2, 0:1],
                                    bias_sb[0:2, FC2_B:FC2_B + 1], None,
                                    op0=ALU.add)
            nc.sync.dma_start(t_out.ap().rearrange("a b -> b a"), lab[:])

    nc.compile()
    return nc


_PROGRAM = None


def _get_program():
    global _PROGRAM
    if _PROGRAM is None:
        _PROGRAM = build_program()
    return _PROGRAM


def _host_inputs(inputs):
    f = {k: np.asarray(v, np.float32) for k, v in inputs.items()}

    def bf(x):
        return np.ascontiguousarray(np.asarray(x, np.float32).astype(BF16))

    bias = np.zeros((128, NCOL), np.float32)

    def put(col, vec):
        v = np.asarray(vec, np.float32).reshape(-1)
        for c in range(len(v) // 128):
            bias[:, col + c] = v[128 * c:128 * c + 128]

    put(FT_B, f['ft_b'])
    for l in range(L):
        put(_bcol(l, SA_BQ), f['sa_bq'][l])
        put(_bcol(l, SA_BO), f['sa_bv'][l] @ f['sa_wo'][l] + f['sa_bo'][l])
        put(_bcol(l, EA_BQ), f['ea_bq'][l])
        put(_bcol(l, EA_BO), f['ea_bv'][l] @ f['ea_wo'][l] + f['ea_bo'][l])
        put(_bcol(l, B1), f['pf_b1'][l])
        put(_bcol(l, B2), f['pf_b2'][l])
        put(_bcol(l, LNG), f['ln_g'][l])
        put(_bcol(l, LNB), f['ln_b'][l])
    put(FC1_B, f['fc1_b'])
    bias[0:2, FC2_B] = f['fc2_b']

    shared = {'ftw': bf(f['ft_w']), 'bias': bias,
              'fc1': bf(f['fc1_w']), 'fc2': bf(f['fc2_w'])}
    for l in range(L):
        shared[f'saq{l}'] = bf(f['sa_wq'][l])
        shared[f'sak{l}'] = bf(f['sa_wk'][l])
        shared[f'sav{l}'] = bf(f['sa_wv'][l])
        shared[f'sao{l}'] = bf(f['sa_wo'][l])
        shared[f'eaq{l}'] = bf(f['ea_wq'][l])
        shared[f'eak{l}'] = bf(f['ea_wk'][l])
        shared[f'eav{l}'] = bf(f['ea_wv'][l])
        shared[f'eao{l}'] = bf(f['ea_wo'][l])
        shared[f'w1_{l}'] = bf(f['pf_w1'][l])
        shared[f'w2_{l}'] = bf(f['pf_w2'][l])

    in_maps = []
    for i in range(C):
        sl = slice(TC * i, TC * (i + 1))
        m = dict(shared)
        m['trgT'] = bf(f['trg'][0, sl, :].T)
        m['srcT'] = bf(f['src'][0, sl, :].T)
        in_maps.append(m)
    return in_maps


def kernel(**inputs):
    import os
    nc = _get_program()
    in_maps = _host_inputs(inputs)
    trace = bool(int(os.environ.get("KERNEL_TRACE", "0")))
    res = bass_utils.run_bass_kernel_spmd(
        nc, in_maps, core_ids=list(range(C)), trace=trace)
    if trace:
        kernel.last_exec_time_ns = res.exec_time_ns
    return np.asarray(res.results[0]["out"], np.float32)
